# revision 13
# baseline (speedup 1.0000x reference)
"""GAT layer (nn_GAT) on 8 Trainium2 NeuronCores — Bass/Tile SPMD kernel.

Math (per head h):
    Wh   = x @ W[h]                         [N, HID]
    s_i  = Wh_i . a1[h],  d_j = Wh_j . a2[h]
    e_ij = leakyrelu(s_i + d_j, 0.2), masked by adj, softmax over j
    out  = elu(att @ Wh)

Two on-device routes produce the same masked softmax numerator/denominator
(one PE matmul against rhs = [Wh | 1] accumulates both):

DVE route (per j-chunk, head):
    exp(LR(s_i + d_j)) = max(e^z, e^{az})  (a = 0.2)
    p_ji = adj01_ji * max(w_i * A_j, B_j)   with w = e^{(1-a)s - 10ln2},
    A = e^d, B = e^{ad - 10ln2} — one fused TensorScalar (mult+max) plus one
    tensor_tensor mult per tile, then matmuls p @ rhs.  The row factor
    e^{a s_i} cancels in the softmax; the 2^-10 scale keeps fp16 safe.

SPLIT route (first NSPLIT j-chunks): the leaky-relu branch is resolved on the
host into a per-head sign mask maskA = adj AND (s_i + d_j >= 0) (fp8 {0,1});
then exactly
    sum_j p_ji rhs_jc = w_i * MM(maskA, A (.) rhs)          -> ACC_A
                        - MM(maskA, B (.) rhs) + MM(adj01, B (.) rhs) -> main
so those chunks cost three small matmul chains and no N^2 DVE work at all.
The epilogue adds w_i (.) ACC_A into the main accumulator before the softmax
division.  A few DVE-route mask-multiplies can be farmed to GpSimd (GP_EVERY).

Sharding: 8 cores = 2 head-groups x 4 row-groups; per-core inputs are
column-permuted so the core's own rows come first (SPMD-identical program).
elu(v) = relu(v) + min(exp(v), 1) - 1.
"""

from contextlib import ExitStack

import numpy as np

import concourse.bass as bass
import concourse.bacc as bacc
import concourse.mybir as mybir
import concourse.tile as tile
from concourse.bass_utils import run_bass_kernel_spmd
from concourse.masks import make_identity

N, F, HID, H = 4096, 512, 64, 8
ALPHA = 0.2
HG, RG = 2, 4                 # head groups x row groups
HPC, RPC = H // HG, N // RG   # 4 heads / 1024 rows per core
NB = N // 128                 # 32 projection row-blocks == j-chunks
FB = F // 128                 # 4 contraction chunks
IB = RPC // 128               # 8 output row-blocks per core
NCORES = 8
SC_BIAS = -float(np.log(1024.0))  # exp(z + SC_BIAS) = exp(z) / 1024

NSPLIT = 8     # leading j-chunks handled by the matmul (sign-mask) route
GP_EVERY = 3   # every GP_EVERY-th DVE mask-mult pair goes to GpSimd (0 = off)
NDVE = NB - NSPLIT

_CACHE = {}


def _build():
    f8, f16, f32 = mybir.dt.float8e4, mybir.dt.float16, mybir.dt.float32
    Alu = mybir.AluOpType
    Act = mybir.ActivationFunctionType

    nc = bacc.Bacc()
    # all inputs are host-swizzled so each SBUF partition reads one long
    # contiguous DRAM run (big DMA descriptors)
    xhi = nc.declare_dram_parameter("xhi", [128, FB, N], f16, isOutput=False)
    # [ W(4 heads, 64 each) | wsd(8) ]
    wext = nc.declare_dram_parameter("wext", [128, FB, 264], f16, isOutput=False)
    adjt = nc.declare_dram_parameter("adjt", [128, NDVE, RPC], f16, isOutput=False)
    out_t = nc.declare_dram_parameter("out", [HPC, RPC, HID], f32, isOutput=True)
    if NSPLIT:
        maska = nc.declare_dram_parameter(
            "maska", [128, NSPLIT, HPC, RPC], f8, isOutput=False
        )
        maskg = nc.declare_dram_parameter(
            "maskg", [128, NSPLIT, RPC], f8, isOutput=False
        )

    with ExitStack() as ctx:
        tc = ctx.enter_context(tile.TileContext(nc))
        const = ctx.enter_context(tc.tile_pool(name="const", bufs=1))
        ppool = ctx.enter_context(tc.tile_pool(name="ppool", bufs=3))
        epool = ctx.enter_context(tc.tile_pool(name="epool", bufs=4))
        opool = ctx.enter_context(tc.tile_pool(name="opool", bufs=4))

        wext_sb = const.tile([128, FB, 264], f16)
        xhi_sb = const.tile([128, FB, N], f16)
        adjt_tiles = [
            const.tile([128, 4, RPC], f16, name=f"adjt{g}") for g in range(NDVE // 4)
        ]
        if NSPLIT:
            maska_sb = const.tile([128, NSPLIT, HPC, RPC], f8)
            maskg_sb = const.tile([128, NSPLIT, RPC], f8)
            rhsA_sb = const.tile([128, NSPLIT, HPC, 65], f16)
            rhsB_sb = const.tile([128, NSPLIT, HPC, 65], f16)
            rhsBn_sb = const.tile([128, NSPLIT, HPC, 65], f16)
        rhs_sb = const.tile([128, HPC, NB, 65], f16)
        sd_sb = const.tile([128, NB, 8], f32)
        A_sb = const.tile([128, NB, HPC], f32)
        B_sb = const.tile([128, NB, HPC], f32)
        wcol_sb = const.tile([128, IB, HPC], f16)
        wcol32_sb = const.tile([128, IB, HPC], f32)
        ident_sb = const.tile([128, 128], f16)
        wct_sb = const.tile([32, 128], f16)
        onehot_sb = const.tile([32, IB * HPC, 128], f16)
        wbc_tiles = [
            const.tile([128, RPC], f16, name=f"wbc{h}") for h in range(HPC)
        ]
        scbias = const.tile([128, 1], f32)
        nc.vector.memset(scbias, SC_BIAS)
        nc.vector.memset(onehot_sb[:, :, :], 0.0)
        # onehot[k, r, m] = 1 iff k == r (selector rows for the w broadcast)
        nc.gpsimd.affine_select(
            out=onehot_sb[:, :, :], in_=onehot_sb[:, :, :],
            compare_op=Alu.not_equal, fill=1.0, base=0,
            pattern=[[-1, IB * HPC], [0, 128]], channel_multiplier=1,
        )
        make_identity(nc, ident_sb[:, :])

        # DMA issue order: x first (the projection gates everything), then the
        # split-route masks (consumed from chunk 0), then the DVE-route
        # adjacency in ascending chunk order.
        nc.sync.dma_start(out=wext_sb[:, :, :], in_=wext[:, :, :])
        HN = N // 2
        for f in range(FB):
            nc.sync.dma_start(out=xhi_sb[:, f, 0:HN], in_=xhi[:, f, 0:HN])
            nc.sync.dma_start(out=xhi_sb[:, f, HN:N], in_=xhi[:, f, HN:N])
        if NSPLIT:
            for s in range(NSPLIT):
                nc.sync.dma_start(
                    out=maska_sb[:, s, :, :], in_=maska[:, s, :, :]
                )
            nc.sync.dma_start(out=maskg_sb[:, :, :], in_=maskg[:, :, :])
        for g in range(NDVE // 4):
            nc.sync.dma_start(
                out=adjt_tiles[g][:, :, :], in_=adjt[:, 4 * g:4 * (g + 1), :]
            )
        # softmax-denominator ones column of the matmul rhs
        nc.vector.memset(rhs_sb[:, :, :, 64:65], 1.0)

        psmain = ctx.enter_context(tc.tile_pool(name="psmain", bufs=1, space="PSUM"))
        pswide = tc.alloc_tile_pool(name="pswide", bufs=2, space="PSUM")

        def alloc_head_psums(h):
            # 4 row-blocks packed per PSUM bank; start=True clears the whole
            # bank's has_written bits, so only the bank's first matmul sets it
            P0 = psmain.tile([128, 4, 65], f32, tag="P0", name=f"P0_{h}")
            P1 = psmain.tile([128, 4, 65], f32, tag="P1", name=f"P1_{h}")
            if not NSPLIT:
                return P0, P1, None, None
            A0 = psmain.tile([128, 4, 65], f32, tag="A0", name=f"A0_{h}")
            A1 = psmain.tile([128, 4, 65], f32, tag="A1", name=f"A1_{h}")
            return P0, P1, A0, A1

        def emit_split(h, jc, P0, P1, A0, A1):
            # three matmul chains, no N^2 elementwise work:
            #   ACC_A  += maskA @ (A (.) rhs)      (epilogue scales by w_i)
            #   main   += maskA @ (-B (.) rhs) + adj01 @ (B (.) rhs)
            for ib in range(IB):
                A = (A0 if ib < 4 else A1)[:, ib % 4, :]
                P = (P0 if ib < 4 else P1)[:, ib % 4, :]
                ma = maska_sb[:, jc, h, ib * 128:(ib + 1) * 128]
                nc.tensor.matmul(
                    A, ma, rhsA_sb[:, jc, h, :],
                    start=(jc == 0 and ib % 4 == 0),
                    stop=(jc == NSPLIT - 1 and ib % 4 == 3),
                    skip_group_check=True,
                )
                nc.tensor.matmul(
                    P, ma, rhsBn_sb[:, jc, h, :],
                    start=False, stop=False, skip_group_check=True,
                )
                nc.tensor.matmul(
                    P, maskg_sb[:, jc, ib * 128:(ib + 1) * 128],
                    rhsB_sb[:, jc, h, :],
                    start=False,
                    stop=(jc == NSPLIT - 1 and ib % 4 == 3),
                    skip_group_check=True,
                )

        pair_ctr = [0]

        def emit_pair(h, jc0, P0, P1):
            # two j-chunks per mask op: TS scalars differ per chunk so the
            # fused mult+max stays per-chunk; the {0,1} mask multiply runs
            # once over [128, 2*RPC], halving its fixed overhead.
            g, r = (jc0 - NSPLIT) // 4, (jc0 - NSPLIT) % 4
            t = ppool.tile([128, 2, RPC], f16, tag="t", name=f"t_{h}_{jc0}")
            for q in range(2):
                jc = jc0 + q
                nc.vector.tensor_scalar(
                    t[:, q, :], wbc_tiles[h][:, :],
                    A_sb[:, jc, h:h + 1], B_sb[:, jc, h:h + 1],
                    Alu.mult, Alu.max,
                )
            p = ppool.tile([128, 2, RPC], f16, tag="p", name=f"p_{h}_{jc0}")
            pair_ctr[0] += 1
            eng = nc.vector
            if GP_EVERY and pair_ctr[0] % GP_EVERY == 0:
                eng = nc.gpsimd
            eng.tensor_tensor(
                p[:, :, :], t[:, :, :], adjt_tiles[g][:, r:r + 2, :], Alu.mult
            )
            for q in range(2):
                jc = jc0 + q
                for ib in range(IB):
                    P = P0 if ib < 4 else P1
                    nc.tensor.matmul(
                        P[:, ib % 4, :], p[:, q, ib * 128:(ib + 1) * 128],
                        rhs_sb[:, h, jc, :],
                        start=(jc == NSPLIT and ib % 4 == 0),
                        stop=(NSPLIT == 0 and jc == NB - 1 and ib % 4 == 3),
                        skip_group_check=True,
                    )

        def emit_epilogue(h, P0, P1, A0, A1):
            for ib in range(IB):
                P = (P0 if ib < 4 else P1)[:, ib % 4, :]
                if NSPLIT:
                    A = (A0 if ib < 4 else A1)[:, ib % 4, :]
                    tmp = epool.tile([128, 65], f32, tag="tmp", name=f"tm{h}_{ib}")
                    nc.vector.tensor_scalar(
                        tmp, A, wcol32_sb[:, ib, h:h + 1], None, Alu.mult
                    )
                    pre = epool.tile([128, 65], f32, tag="pre", name=f"pr{h}_{ib}")
                    nc.vector.tensor_tensor(pre, P, tmp, Alu.add)
                    P = pre
                rcp = epool.tile([128, 1], f32, tag="rcp", name=f"rcp{h}_{ib}")
                nc.vector.reciprocal(rcp, P[:, 64:65])
                rt = epool.tile([128, 64], f32, tag="rt", name=f"rt{h}_{ib}")
                nc.scalar.activation(rt, P[:, 0:64], Act.Relu, scale=rcp)
                qt = epool.tile([128, 64], f32, tag="qt", name=f"qt{h}_{ib}")
                nc.scalar.activation(qt, P[:, 0:64], Act.Exp, scale=rcp)
                # elu(v) = relu(v) - relu(1 - e^v)
                mt = epool.tile([128, 64], f32, tag="mt", name=f"mt{h}_{ib}")
                nc.scalar.activation(mt, qt, Act.Relu, scale=-1.0, bias=1.0)
                ot = opool.tile([128, 64], f32, tag="ot", name=f"ot{h}_{ib}")
                nc.vector.tensor_tensor(ot, rt, mt, Alu.subtract)
                nc.sync.dma_start(out=out_t[h, ib * 128:(ib + 1) * 128, :], in_=ot)

        def emit_wbc(h):
            # broadcast head h's w row to all 128 partitions via one-hot matmuls
            for b in range(IB):
                r = b * HPC + h
                bc_ps = pswide.tile([128, 128], f32, tag="bcps", bufs=1)
                nc.tensor.matmul(
                    bc_ps, onehot_sb[:, r, :], wct_sb[:, :], start=True, stop=True
                )
                nc.scalar.activation(
                    wbc_tiles[h][:, b * 128:(b + 1) * 128], bc_ps, Act.Copy
                )

        def emit_rhs_scaled(nb):
            # split-route rhs variants: A(.)rhs, B(.)rhs, -B(.)rhs
            for h in range(HPC):
                nc.vector.tensor_scalar(
                    rhsA_sb[:, nb, h, :], rhs_sb[:, h, nb, :],
                    A_sb[:, nb, h:h + 1], None, Alu.mult,
                )
                nc.vector.tensor_scalar(
                    rhsB_sb[:, nb, h, :], rhs_sb[:, h, nb, :],
                    B_sb[:, nb, h:h + 1], None, Alu.mult,
                )
                nc.vector.tensor_scalar(
                    rhsBn_sb[:, nb, h, :], rhs_sb[:, h, nb, :],
                    B_sb[:, nb, h:h + 1], -1.0, Alu.mult, Alu.mult,
                )

        # ---- projection + head-0 attention, interleaved ----
        P0_0, P1_0, A0_0, A1_0 = alloc_head_psums(0)
        for nb in range(NB):
            pw = pswide.tile([128, 264], f32)
            for f in range(FB):
                nc.tensor.matmul(
                    pw, xhi_sb[:, f, nb * 128:(nb + 1) * 128], wext_sb[:, f, 0:264],
                    start=(f == 0), stop=(f == FB - 1),
                )
            # sd first (the w/A/B chain is on the critical path), then rhs
            nc.scalar.activation(sd_sb[:, nb, :], pw[:, 256:264], Act.Copy)
            dcols = sd_sb[:, nb:nb + 1, 1:8:2]
            nc.scalar.activation(A_sb[:, nb:nb + 1, :], dcols, Act.Exp)
            nc.scalar.activation(
                B_sb[:, nb:nb + 1, :], dcols, Act.Exp,
                scale=ALPHA, bias=scbias[:, :],
            )
            nc.scalar.activation(
                rhs_sb[:, :, nb, 0:64],
                pw[:, 0:256].rearrange("p (h d) -> p h d", h=HPC),
                Act.Copy,
            )
            if NSPLIT and nb < NSPLIT:
                emit_rhs_scaled(nb)
            if nb == IB - 1:
                # own rows (blocks 0..7 thanks to the permutation): w factors,
                # transpose to one free-dim row — no DMA
                nc.scalar.activation(
                    wcol_sb[:, :, :], sd_sb[:, 0:IB, 0:8:2],
                    Act.Exp, scale=1.0 - ALPHA, bias=scbias[:, :],
                )
                if NSPLIT:
                    nc.scalar.activation(
                        wcol32_sb[:, :, :], sd_sb[:, 0:IB, 0:8:2],
                        Act.Exp, scale=1.0 - ALPHA, bias=scbias[:, :],
                    )
                wct_ps = pswide.tile([32, 128], f16, tag="wctps", bufs=1)
                nc.tensor.transpose(
                    wct_ps, wcol_sb[:, :, :].rearrange("p a b -> p (a b)"),
                    ident_sb[:, :],
                )
                nc.scalar.activation(wct_sb[:, :], wct_ps[:, :], Act.Copy)
                emit_wbc(0)
            if nb >= IB - 1 and (nb - (IB - 1)) % 5 == 4:
                hh = (nb - (IB - 1)) // 5 + 1
                if hh < HPC:
                    emit_wbc(hh)
            # head-0 DVE pairs, trailing the projection
            if nb >= IB + 1 and nb % 2 == 1:
                jc = NSPLIT + (nb - IB - 1)
                if jc + 1 < NB:
                    emit_pair(0, jc, P0_0, P1_0)
        pswide.release()
        # pairs not already emitted inside the projection loop, then the
        # split-route chains (their masks arrive late in the DMA stream)
        for jc in range(NSPLIT + NB - IB, NB, 2):
            emit_pair(0, jc, P0_0, P1_0)
        for jc in range(NSPLIT):
            emit_split(0, jc, P0_0, P1_0, A0_0, A1_0)
        emit_epilogue(0, P0_0, P1_0, A0_0, A1_0)

        # ---- remaining heads ----
        for h in range(1, HPC):
            P0, P1, A0, A1 = alloc_head_psums(h)
            for jc in range(NSPLIT, NB, 2):
                emit_pair(h, jc, P0, P1)
            for jc in range(NSPLIT):
                emit_split(h, jc, P0, P1, A0, A1)
            emit_epilogue(h, P0, P1, A0, A1)
    nc.finalize()
    return nc


def _get_nc():
    if "nc" not in _CACHE:
        _CACHE["nc"] = _build()
    return _CACHE["nc"]


def _prepare_in_maps(x, adj, W, a):
    import ml_dtypes

    f8 = ml_dtypes.float8_e4m3
    x = np.asarray(x, np.float32)
    adj = np.asarray(adj, np.float32)
    W = np.asarray(W, np.float32)
    a = np.asarray(a, np.float32)
    xT = np.ascontiguousarray(x.T)
    adjT = np.ascontiguousarray(adj.T)
    # attention-logit projections: s = x @ (W a1), d = x @ (W a2)  [N, H]
    wa1 = np.einsum("hfd,hd->fh", W, a[:, :HID])
    wa2 = np.einsum("hfd,hd->fh", W, a[:, HID:])
    s_all = x @ wa1
    d_all = x @ wa2
    all_rows = np.arange(N)
    in_maps = []

    def swz(m):
        # [K*128, M] -> [128, K, M]: partition-major so each SBUF
        # partition reads one contiguous DRAM run
        k = m.shape[0] // 128
        return np.ascontiguousarray(
            m.reshape(k, 128, *m.shape[1:]).transpose(1, 0, *range(2, m.ndim + 1))
        )

    for c in range(NCORES):
        hg, rg = divmod(c, RG)
        own = np.arange(rg * RPC, (rg + 1) * RPC)
        perm = np.concatenate([own, np.delete(all_rows, own)])
        xt = xT[:, perm]
        xhi = xt.astype(np.float16)
        heads = [hg * HPC + h for h in range(HPC)]
        wsd = np.stack(
            sum([[wa1[:, gh], wa2[:, gh]] for gh in heads], []), axis=1
        ).astype(np.float16)  # [F, 8] cols (h0 s, h0 d, h1 s, ...)
        wext = np.concatenate(
            [W[gh] for gh in heads] + [wsd], axis=1
        ).astype(np.float16)  # [F, 264]
        adj01 = adjT[perm][:, own]  # [N j, RPC i] in {0,1}
        m = {
            "xhi": swz(xhi),
            "wext": swz(wext),
            "adjt": swz(adj01[NSPLIT * 128:].astype(np.float16)),
        }
        if NSPLIT:
            js = perm[: NSPLIT * 128]
            # maskA[j, h, i] = adj AND (s_i + d_j >= 0)
            za = (
                d_all[js][:, heads, None] + s_all[own][:, heads].T[None, :, :]
            ) >= 0.0  # [NSPLIT*128, HPC, RPC]
            adj_s = adj01[: NSPLIT * 128]
            m["maska"] = swz(
                (za & (adj_s[:, None, :] > 0)).astype(f8)
            )
            m["maskg"] = swz(adj_s.astype(f8))
        in_maps.append(m)
    return in_maps


def _assemble(results):
    full = np.empty((N, H * HID), np.float32)
    for c in range(NCORES):
        hg, rg = divmod(c, RG)
        o = results[c]["out"]  # [HPC, RPC, HID]
        full[rg * RPC:(rg + 1) * RPC, hg * HPC * HID:(hg + 1) * HPC * HID] = (
            o.transpose(1, 0, 2).reshape(RPC, HPC * HID)
        )
    return full


def _run(in_maps, **kw):
    return run_bass_kernel_spmd(_get_nc(), in_maps, list(range(NCORES)), **kw)


def kernel(x, adj, W, a):
    in_maps = _prepare_in_maps(x, adj, W, a)
    res = _run(in_maps)
    return _assemble(res.results)


# revision 16
# speedup vs baseline: 1.0272x; 1.0272x over previous
"""GAT layer (nn_GAT) on 8 Trainium2 NeuronCores — Bass/Tile SPMD kernel.

Math (per head h):
    Wh   = x @ W[h]                         [N, HID]
    s_i  = Wh_i . a1[h],  d_j = Wh_j . a2[h]
    e_ij = leakyrelu(s_i + d_j, 0.2), masked by adj, softmax over j
    out  = elu(att @ Wh)

Two on-device routes produce the same masked softmax numerator/denominator
(one PE matmul against rhs = [Wh | 1] accumulates both):

DVE route (per j-chunk, head):
    exp(LR(s_i + d_j)) = max(e^z, e^{az})  (a = 0.2)
    p_ji = adj01_ji * max(w_i * A_j, B_j)   with w = e^{(1-a)s - 10ln2},
    A = e^d, B = e^{ad - 10ln2} — one fused TensorScalar (mult+max) plus one
    tensor_tensor mult per tile, then matmuls p @ rhs.  The row factor
    e^{a s_i} cancels in the softmax; the 2^-10 scale keeps fp16 safe.

SPLIT route (first NSPLIT j-chunks): the leaky-relu branch is resolved on the
host into a per-head sign mask maskA = adj AND (s_i + d_j >= 0) (fp8 {0,1});
then exactly
    sum_j p_ji rhs_jc = w_i * MM(maskA, A (.) rhs)          -> ACC_A
                        - MM(maskA, B (.) rhs) + MM(adj01, B (.) rhs) -> main
so those chunks cost three small matmul chains and no N^2 DVE work at all.
The epilogue adds w_i (.) ACC_A into the main accumulator before the softmax
division.  A few DVE-route mask-multiplies can be farmed to GpSimd (GP_EVERY).

Sharding: 8 cores = 2 head-groups x 4 row-groups; per-core inputs are
column-permuted so the core's own rows come first (SPMD-identical program).
elu(v) = relu(v) + min(exp(v), 1) - 1.
"""

from contextlib import ExitStack

import numpy as np

import concourse.bass as bass
import concourse.bacc as bacc
import concourse.mybir as mybir
import concourse.tile as tile
from concourse.bass_utils import run_bass_kernel_spmd
from concourse.masks import make_identity

N, F, HID, H = 4096, 512, 64, 8
ALPHA = 0.2
HG, RG = 2, 4                 # head groups x row groups
HPC, RPC = H // HG, N // RG   # 4 heads / 1024 rows per core
NB = N // 128                 # 32 projection row-blocks == j-chunks
FB = F // 128                 # 4 contraction chunks
IB = RPC // 128               # 8 output row-blocks per core
NCORES = 8
SC_BIAS = -float(np.log(1024.0))  # exp(z + SC_BIAS) = exp(z) / 1024

NSPLIT = 8     # leading j-chunks handled by the matmul (sign-mask) route
GP_EVERY = 3   # every GP_EVERY-th DVE mask-mult pair goes to GpSimd (0 = off)
NDVE = NB - NSPLIT

_CACHE = {}


def _build():
    f8, f16, f32 = mybir.dt.float8e4, mybir.dt.float16, mybir.dt.float32
    Alu = mybir.AluOpType
    Act = mybir.ActivationFunctionType

    nc = bacc.Bacc()
    # all inputs are host-swizzled so each SBUF partition reads one long
    # contiguous DRAM run (big DMA descriptors)
    xhi = nc.declare_dram_parameter("xhi", [128, FB, N], f16, isOutput=False)
    # [ W(4 heads, 64 each) | wsd(8) ]
    wext = nc.declare_dram_parameter("wext", [128, FB, 264], f16, isOutput=False)
    adjt = nc.declare_dram_parameter("adjt", [128, NDVE, RPC], f16, isOutput=False)
    out_t = nc.declare_dram_parameter("out", [HPC, RPC, HID], f32, isOutput=True)
    if NSPLIT:
        maskaa = nc.declare_dram_parameter(
            "maskaa", [128, NSPLIT, HPC, RPC], f8, isOutput=False
        )
        maskbb = nc.declare_dram_parameter(
            "maskbb", [128, NSPLIT, HPC, RPC], f8, isOutput=False
        )

    with ExitStack() as ctx:
        tc = ctx.enter_context(tile.TileContext(nc))
        const = ctx.enter_context(tc.tile_pool(name="const", bufs=1))
        ppool = ctx.enter_context(tc.tile_pool(name="ppool", bufs=2))
        epool = ctx.enter_context(tc.tile_pool(name="epool", bufs=2))
        opool = ctx.enter_context(tc.tile_pool(name="opool", bufs=4))

        wext_sb = const.tile([128, FB, 264], f16)
        xhi_sb = const.tile([128, FB, N], f16)
        adjt_tiles = [
            const.tile([128, 4, RPC], f16, name=f"adjt{g}") for g in range(NDVE // 4)
        ]
        if NSPLIT:
            maskaa_sb = const.tile([128, NSPLIT, HPC, RPC], f8)
            maskbb_sb = const.tile([128, NSPLIT, HPC, RPC], f8)
            rhs2_sb = const.tile([128, HPC, NSPLIT, 65], f16)
        rhs_sb = const.tile([128, HPC, NB, 65], f16)
        sd_sb = const.tile([128, NB, 8], f32)
        A_sb = const.tile([128, NB, HPC], f32)
        B_sb = const.tile([128, NB, HPC], f32)
        wcol_sb = const.tile([128, IB, HPC], f16)
        wcol32_sb = const.tile([128, IB, HPC], f32)
        ident_sb = const.tile([128, 128], f16)
        wct_sb = const.tile([32, 128], f16)
        onehot_sb = const.tile([32, IB * HPC, 128], f16)
        wbc_tiles = [
            const.tile([128, RPC], f16, name=f"wbc{h}") for h in range(HPC)
        ]
        scbias = const.tile([128, 1], f32)
        nc.vector.memset(scbias, SC_BIAS)
        scbias5 = const.tile([128, 1], f32)
        nc.vector.memset(scbias5, SC_BIAS + 5.0 * float(np.log(2.0)))
        nc.vector.memset(onehot_sb[:, :, :], 0.0)
        # onehot[k, r, m] = 1 iff k == r (selector rows for the w broadcast)
        nc.gpsimd.affine_select(
            out=onehot_sb[:, :, :], in_=onehot_sb[:, :, :],
            compare_op=Alu.not_equal, fill=1.0, base=0,
            pattern=[[-1, IB * HPC], [0, 128]], channel_multiplier=1,
        )
        make_identity(nc, ident_sb[:, :])

        # DMA issue order: x first (the projection gates everything), then the
        # split-route masks (consumed from chunk 0), then the DVE-route
        # adjacency in ascending chunk order.
        nc.sync.dma_start(out=wext_sb[:, :, :], in_=wext[:, :, :])
        HN = N // 2
        for f in range(FB):
            nc.sync.dma_start(out=xhi_sb[:, f, 0:HN], in_=xhi[:, f, 0:HN])
            nc.sync.dma_start(out=xhi_sb[:, f, HN:N], in_=xhi[:, f, HN:N])
        if NSPLIT:
            for s in range(NSPLIT):
                nc.sync.dma_start(
                    out=maskaa_sb[:, s, :, :], in_=maskaa[:, s, :, :]
                )
                nc.sync.dma_start(
                    out=maskbb_sb[:, s, :, :], in_=maskbb[:, s, :, :]
                )
        for g in range(NDVE // 4):
            nc.sync.dma_start(
                out=adjt_tiles[g][:, :, :], in_=adjt[:, 4 * g:4 * (g + 1), :]
            )
        # softmax-denominator ones column of the matmul rhs
        nc.vector.memset(rhs_sb[:, :, :, 64:65], 1.0)

        psmain = ctx.enter_context(tc.tile_pool(name="psmain", bufs=1, space="PSUM"))
        pswide = tc.alloc_tile_pool(name="pswide", bufs=2, space="PSUM")

        def alloc_head_psums(h):
            # 4 row-blocks packed per PSUM bank; start=True clears the whole
            # bank's has_written bits, so only the bank's first matmul sets it
            P0 = psmain.tile([128, 4, 65], f32, tag="P0", name=f"P0_{h}")
            P1 = psmain.tile([128, 4, 65], f32, tag="P1", name=f"P1_{h}")
            if not NSPLIT:
                return P0, P1, None, None
            A0 = psmain.tile([128, 4, 65], f32, tag="A0", name=f"A0_{h}")
            A1 = psmain.tile([128, 4, 65], f32, tag="A1", name=f"A1_{h}")
            return P0, P1, A0, A1

        def emit_split(h, jc, P0, P1, A0, A1):
            # two matmul chains, no N^2 elementwise work (A/B are folded into
            # the host-shipped fp8 masks):
            #   ACC_A += (maskA.A.2^-5) @ rhs      (epilogue scales by w_i 2^5)
            #   main  += (maskB.B.2^8) @ (rhs 2^-8)
            for ib in range(IB):
                A = (A0 if ib < 4 else A1)[:, ib % 4, :]
                P = (P0 if ib < 4 else P1)[:, ib % 4, :]
                nc.tensor.matmul(
                    A, maskaa_sb[:, jc, h, ib * 128:(ib + 1) * 128],
                    rhs_sb[:, h, jc, :],
                    start=(jc == 0 and ib % 4 == 0),
                    stop=(jc == NSPLIT - 1 and ib % 4 == 3),
                    skip_group_check=True,
                )
                nc.tensor.matmul(
                    P, maskbb_sb[:, jc, h, ib * 128:(ib + 1) * 128],
                    rhs2_sb[:, h, jc, :],
                    start=False,
                    stop=(jc == NSPLIT - 1 and ib % 4 == 3),
                    skip_group_check=True,
                )

        pair_ctr = [0]

        def emit_pair(h, jc0, P0, P1):
            # two j-chunks per mask op: TS scalars differ per chunk so the
            # fused mult+max stays per-chunk; the {0,1} mask multiply runs
            # once over [128, 2*RPC], halving its fixed overhead.
            g, r = (jc0 - NSPLIT) // 4, (jc0 - NSPLIT) % 4
            t = ppool.tile([128, 2, RPC], f16, tag="t", name=f"t_{h}_{jc0}")
            for q in range(2):
                jc = jc0 + q
                nc.vector.tensor_scalar(
                    t[:, q, :], wbc_tiles[h][:, :],
                    A_sb[:, jc, h:h + 1], B_sb[:, jc, h:h + 1],
                    Alu.mult, Alu.max,
                )
            p = ppool.tile([128, 2, RPC], f16, tag="p", name=f"p_{h}_{jc0}")
            pair_ctr[0] += 1
            eng = nc.vector
            if GP_EVERY and pair_ctr[0] % GP_EVERY == 0:
                eng = nc.gpsimd
            eng.tensor_tensor(
                p[:, :, :], t[:, :, :], adjt_tiles[g][:, r:r + 2, :], Alu.mult
            )
            for q in range(2):
                jc = jc0 + q
                for ib in range(IB):
                    P = P0 if ib < 4 else P1
                    nc.tensor.matmul(
                        P[:, ib % 4, :], p[:, q, ib * 128:(ib + 1) * 128],
                        rhs_sb[:, h, jc, :],
                        start=(jc == NSPLIT and ib % 4 == 0),
                        stop=(NSPLIT == 0 and jc == NB - 1 and ib % 4 == 3),
                        skip_group_check=True,
                    )

        def emit_epilogue(h, P0, P1, A0, A1):
            for half in range(2):
                Ph = P0 if half == 0 else P1
                src_t = Ph
                if NSPLIT:
                    Ah = A0 if half == 0 else A1
                    wA = epool.tile([128, 4, 65], f32, tag="wA", name=f"wA{h}_{half}")
                    for k in range(4):
                        ib = half * 4 + k
                        nc.scalar.activation(
                            wA[:, k, :], Ah[:, k, :], Act.Copy,
                            scale=wcol32_sb[:, ib, h:h + 1],
                        )
                    pre = epool.tile([128, 4, 65], f32, tag="pre", name=f"pre{h}_{half}")
                    nc.vector.tensor_tensor(pre, Ph, wA, Alu.add)
                    src_t = pre
                for k in range(4):
                    ib = half * 4 + k
                    P = src_t[:, k, :]
                    rcp = epool.tile([128, 1], f32, tag="rcp", name=f"rcp{h}_{ib}")
                    nc.vector.reciprocal(rcp, P[:, 64:65])
                    rt = epool.tile([128, 64], f32, tag="rt", name=f"rt{h}_{ib}")
                    nc.scalar.activation(rt, P[:, 0:64], Act.Relu, scale=rcp)
                    qt = epool.tile([128, 64], f32, tag="qt", name=f"qt{h}_{ib}")
                    nc.scalar.activation(qt, P[:, 0:64], Act.Exp, scale=rcp)
                    # elu(v) = relu(v) - relu(1 - e^v)
                    mt = epool.tile([128, 64], f32, tag="mt", name=f"mt{h}_{ib}")
                    nc.scalar.activation(mt, qt, Act.Relu, scale=-1.0, bias=1.0)
                    ot = opool.tile([128, 64], f32, tag="ot", name=f"ot{h}_{ib}")
                    nc.vector.tensor_tensor(ot, rt, mt, Alu.subtract)
                    nc.sync.dma_start(
                        out=out_t[h, ib * 128:(ib + 1) * 128, :], in_=ot
                    )

        def emit_wbc(h):
            # broadcast head h's w row to all 128 partitions via one-hot matmuls
            for b in range(IB):
                r = b * HPC + h
                bc_ps = pswide.tile([128, 128], f32, tag="bcps", bufs=1)
                nc.tensor.matmul(
                    bc_ps, onehot_sb[:, r, :], wct_sb[:, :], start=True, stop=True
                )
                nc.scalar.activation(
                    wbc_tiles[h][:, b * 128:(b + 1) * 128], bc_ps, Act.Copy
                )

        def emit_rhs_scaled(nb):
            # 2^-8-scaled rhs copy for the maskBB chain (all heads at once)
            nc.vector.tensor_scalar(
                rhs2_sb[:, :, nb, :], rhs_sb[:, :, nb, :],
                2.0 ** -8, None, Alu.mult,
            )

        # ---- projection + head-0 attention, interleaved ----
        P0_0, P1_0, A0_0, A1_0 = alloc_head_psums(0)
        for nb in range(NB):
            pw = pswide.tile([128, 264], f32)
            for f in range(FB):
                nc.tensor.matmul(
                    pw, xhi_sb[:, f, nb * 128:(nb + 1) * 128], wext_sb[:, f, 0:264],
                    start=(f == 0), stop=(f == FB - 1),
                )
            # sd first (the w/A/B chain is on the critical path), then rhs
            nc.scalar.activation(sd_sb[:, nb, :], pw[:, 256:264], Act.Copy)
            dcols = sd_sb[:, nb:nb + 1, 1:8:2]
            nc.scalar.activation(A_sb[:, nb:nb + 1, :], dcols, Act.Exp)
            nc.scalar.activation(
                B_sb[:, nb:nb + 1, :], dcols, Act.Exp,
                scale=ALPHA, bias=scbias[:, :],
            )
            nc.scalar.activation(
                rhs_sb[:, :, nb, 0:64],
                pw[:, 0:256].rearrange("p (h d) -> p h d", h=HPC),
                Act.Copy,
            )
            if NSPLIT and nb < NSPLIT:
                emit_rhs_scaled(nb)
            if nb == IB - 1:
                # own rows (blocks 0..7 thanks to the permutation): w factors,
                # transpose to one free-dim row — no DMA
                nc.scalar.activation(
                    wcol_sb[:, :, :], sd_sb[:, 0:IB, 0:8:2],
                    Act.Exp, scale=1.0 - ALPHA, bias=scbias[:, :],
                )
                if NSPLIT:
                    nc.scalar.activation(
                        wcol32_sb[:, :, :], sd_sb[:, 0:IB, 0:8:2],
                        Act.Exp, scale=1.0 - ALPHA, bias=scbias5[:, :],
                    )
                wct_ps = pswide.tile([32, 128], f16, tag="wctps", bufs=1)
                nc.tensor.transpose(
                    wct_ps, wcol_sb[:, :, :].rearrange("p a b -> p (a b)"),
                    ident_sb[:, :],
                )
                nc.scalar.activation(wct_sb[:, :], wct_ps[:, :], Act.Copy)
                emit_wbc(0)
            if nb >= IB - 1 and (nb - (IB - 1)) % 5 == 4:
                hh = (nb - (IB - 1)) // 5 + 1
                if hh < HPC:
                    emit_wbc(hh)
            # head-0 DVE pairs, trailing the projection
            if nb >= IB + 1 and nb % 2 == 1:
                jc = NSPLIT + (nb - IB - 1)
                if jc + 1 < NB:
                    emit_pair(0, jc, P0_0, P1_0)
        pswide.release()
        # pairs not already emitted inside the projection loop, then the
        # split-route chains (their masks arrive late in the DMA stream)
        for jc in range(NSPLIT + NB - IB, NB, 2):
            emit_pair(0, jc, P0_0, P1_0)
        for jc in range(NSPLIT):
            emit_split(0, jc, P0_0, P1_0, A0_0, A1_0)
        emit_epilogue(0, P0_0, P1_0, A0_0, A1_0)

        # ---- remaining heads ----
        for h in range(1, HPC):
            P0, P1, A0, A1 = alloc_head_psums(h)
            for jc in range(NSPLIT, NB, 2):
                emit_pair(h, jc, P0, P1)
            for jc in range(NSPLIT):
                emit_split(h, jc, P0, P1, A0, A1)
            emit_epilogue(h, P0, P1, A0, A1)
    nc.finalize()
    return nc


def _get_nc():
    if "nc" not in _CACHE:
        _CACHE["nc"] = _build()
    return _CACHE["nc"]


def _prepare_in_maps(x, adj, W, a):
    import ml_dtypes

    f8 = ml_dtypes.float8_e4m3
    x = np.asarray(x, np.float32)
    adj = np.asarray(adj, np.float32)
    W = np.asarray(W, np.float32)
    a = np.asarray(a, np.float32)
    xT = np.ascontiguousarray(x.T)
    adjT = np.ascontiguousarray(adj.T)
    # attention-logit projections: s = x @ (W a1), d = x @ (W a2)  [N, H]
    wa1 = np.einsum("hfd,hd->fh", W, a[:, :HID])
    wa2 = np.einsum("hfd,hd->fh", W, a[:, HID:])
    s_all = x @ wa1
    d_all = x @ wa2
    all_rows = np.arange(N)
    in_maps = []

    def swz(m):
        # [K*128, M] -> [128, K, M]: partition-major so each SBUF
        # partition reads one contiguous DRAM run
        k = m.shape[0] // 128
        return np.ascontiguousarray(
            m.reshape(k, 128, *m.shape[1:]).transpose(1, 0, *range(2, m.ndim + 1))
        )

    for c in range(NCORES):
        hg, rg = divmod(c, RG)
        own = np.arange(rg * RPC, (rg + 1) * RPC)
        perm = np.concatenate([own, np.delete(all_rows, own)])
        xt = xT[:, perm]
        xhi = xt.astype(np.float16)
        heads = [hg * HPC + h for h in range(HPC)]
        wsd = np.stack(
            sum([[wa1[:, gh], wa2[:, gh]] for gh in heads], []), axis=1
        ).astype(np.float16)  # [F, 8] cols (h0 s, h0 d, h1 s, ...)
        wext = np.concatenate(
            [W[gh] for gh in heads] + [wsd], axis=1
        ).astype(np.float16)  # [F, 264]
        adj01 = adjT[perm][:, own]  # [N j, RPC i] in {0,1}
        m = {
            "xhi": swz(xhi),
            "wext": swz(wext),
            "adjt": swz(adj01[NSPLIT * 128:].astype(np.float16)),
        }
        if NSPLIT:
            js = perm[: NSPLIT * 128]
            # z[j, h, i] = s_i + d_j; A/B folded into the fp8 masks
            za = (
                d_all[js][:, heads, None] + s_all[own][:, heads].T[None, :, :]
            ) >= 0.0  # [NSPLIT*128, HPC, RPC]
            adj_s = adj01[: NSPLIT * 128][:, None, :] > 0
            Ah = np.exp(d_all[js][:, heads]) * 2.0 ** -5  # [NS*128, HPC]
            Bh = np.exp(ALPHA * d_all[js][:, heads] + SC_BIAS) * 2.0 ** 8
            m["maskaa"] = swz(
                ((za & adj_s) * Ah[:, :, None]).astype(f8)
            )
            m["maskbb"] = swz(
                ((~za & adj_s) * Bh[:, :, None]).astype(f8)
            )
        in_maps.append(m)
    return in_maps


def _assemble(results):
    full = np.empty((N, H * HID), np.float32)
    for c in range(NCORES):
        hg, rg = divmod(c, RG)
        o = results[c]["out"]  # [HPC, RPC, HID]
        full[rg * RPC:(rg + 1) * RPC, hg * HPC * HID:(hg + 1) * HPC * HID] = (
            o.transpose(1, 0, 2).reshape(RPC, HPC * HID)
        )
    return full


def _run(in_maps, **kw):
    return run_bass_kernel_spmd(_get_nc(), in_maps, list(range(NCORES)), **kw)


def kernel(x, adj, W, a):
    in_maps = _prepare_in_maps(x, adj, W, a)
    res = _run(in_maps)
    return _assemble(res.results)


# revision 17
# speedup vs baseline: 1.0881x; 1.0593x over previous
"""GAT layer (nn_GAT) on 8 Trainium2 NeuronCores — Bass/Tile SPMD kernel.

Math (per head h):
    Wh   = x @ W[h]                         [N, HID]
    s_i  = Wh_i . a1[h],  d_j = Wh_j . a2[h]
    e_ij = leakyrelu(s_i + d_j, 0.2), masked by adj, softmax over j
    out  = elu(att @ Wh)

Restructuring used on-device (the key trick):
    exp(LR(z)) = max(e^z, e^{az})                      (a = 0.2 < 1)
    exp(LR(s_i + d_j)) = e^{a s_i} * max(w_i * A_j, B_j)
        with w = e^{(1-a)s}, A = e^{d}, B = e^{a d}
    The row factor e^{a s_i} cancels in the softmax, so the masked
    numerator is   p_ji = adjT_ji * max(w_i A_j, B_j)   — ONE fused
    TensorScalar (mult + max, both per-partition operands) and ONE
    tensor_tensor mask multiply per tile.  numerator and denominator
    come out of a single PE matmul with rhs = [Wh | 1].
    Everything is scaled by 2^-10 (cancels in the softmax ratio) so the
    fp16 N^2 path cannot overflow (max value ~9.2e4/16 << 65504).

Sharding: 8 cores = 2 head-groups x 4 row-groups. Each core owns 4 heads
and 1024 output rows; it computes the full projection for its heads (all
4096 j) and row-parallel attention for its rows.  Per-core inputs are
column-permuted so the core's own rows come first — this keeps the SPMD
program identical across cores (no core-id-dependent addressing).

elu(v) = relu(v) + min(exp(v), 1) - 1.
"""

from contextlib import ExitStack

import numpy as np

import concourse.bass as bass
import concourse.bacc as bacc
import concourse.mybir as mybir
import concourse.tile as tile
from concourse.bass_utils import run_bass_kernel_spmd
from concourse.masks import make_identity

N, F, HID, H = 4096, 512, 64, 8
ALPHA = 0.2
HG, RG = 2, 4                 # head groups x row groups
HPC, RPC = H // HG, N // RG   # 4 heads / 1024 rows per core
NB = N // 128                 # 32 projection row-blocks == j-chunks
FB = F // 128                 # 4 contraction chunks
IB = RPC // 128               # 8 output row-blocks per core
NCORES = 8
SC_BIAS = -float(np.log(1024.0))  # exp(z + SC_BIAS) = exp(z) / 1024

_CACHE = {}


def _build():
    f16, f32 = mybir.dt.float16, mybir.dt.float32
    Alu = mybir.AluOpType
    Act = mybir.ActivationFunctionType

    nc = bacc.Bacc()
    # all inputs are host-swizzled so each SBUF partition reads one long
    # contiguous DRAM run (big DMA descriptors)
    xhi = nc.declare_dram_parameter("xhi", [128, FB, N], f16, isOutput=False)
    xlo = nc.declare_dram_parameter("xlo", [128, FB, N], f16, isOutput=False)
    # [ W(4 heads, 64 each) | wsd_hi(8) | wsd_lo(8) ]
    wext = nc.declare_dram_parameter("wext", [128, FB, 272], f16, isOutput=False)
    adjt = nc.declare_dram_parameter("adjt", [128, NB, RPC], f16, isOutput=False)
    out_t = nc.declare_dram_parameter("out", [HPC, RPC, HID], f32, isOutput=True)

    with ExitStack() as ctx:
        tc = ctx.enter_context(tile.TileContext(nc))
        const = ctx.enter_context(tc.tile_pool(name="const", bufs=1))
        dram = ctx.enter_context(tc.tile_pool(name="dram", bufs=1, space="DRAM"))
        ppool = ctx.enter_context(tc.tile_pool(name="ppool", bufs=4))
        epool = ctx.enter_context(tc.tile_pool(name="epool", bufs=4))
        opool = ctx.enter_context(tc.tile_pool(name="opool", bufs=4))

        wext_sb = const.tile([128, FB, 272], f16)
        xhi_sb = const.tile([128, FB, N], f16)
        xlo_sb = const.tile([128, FB, N], f16)
        adjt_tiles = [
            const.tile([128, 4, RPC], f16, name=f"adjt{g}") for g in range(8)
        ]
        rhs_sb = const.tile([128, HPC, NB, 65], f16)
        sd_sb = const.tile([128, NB, 8], f32)
        A_sb = const.tile([128, NB, HPC], f32)
        B_sb = const.tile([128, NB, HPC], f32)
        wcol_sb = const.tile([128, IB, HPC], f16)
        ident_sb = const.tile([128, 128], f16)
        wct_sb = const.tile([32, 128], f16)
        onehot_sb = const.tile([32, IB * HPC, 128], f16)
        wbc_tiles = [
            const.tile([128, RPC], f16, name=f"wbc{h}") for h in range(HPC)
        ]
        scbias = const.tile([128, 1], f32)
        nc.vector.memset(scbias, SC_BIAS)
        nc.vector.memset(onehot_sb[:, :, :], 0.0)
        # onehot[k, r, m] = 1 iff k == r (selector rows for the w broadcast)
        nc.gpsimd.affine_select(
            out=onehot_sb[:, :, :], in_=onehot_sb[:, :, :],
            compare_op=Alu.not_equal, fill=1.0, base=0,
            pattern=[[-1, IB * HPC], [0, 128]], channel_multiplier=1,
        )
        make_identity(nc, ident_sb[:, :])

        # DMA issue order matters: HWDGE queues drain roughly in issue order.
        # The projection gates everything (A/B/w and the matmul rhs all come
        # from it), so x goes first; the main loop then consumes adjacency
        # chunks in ascending jc order at ~1us/chunk, staying just behind the
        # DMA stream.
        nc.sync.dma_start(out=wext_sb[:, :, :], in_=wext[:, :, :])
        HN = N // 2
        for f in range(FB):
            nc.sync.dma_start(out=xhi_sb[:, f, 0:HN], in_=xhi[:, f, 0:HN])
            nc.sync.dma_start(out=xhi_sb[:, f, HN:N], in_=xhi[:, f, HN:N])
        for f in range(FB):
            nc.sync.dma_start(out=xlo_sb[:, f, 0:HN], in_=xlo[:, f, 0:HN])
            nc.sync.dma_start(out=xlo_sb[:, f, HN:N], in_=xlo[:, f, HN:N])
        for g in range(8):
            nc.sync.dma_start(
                out=adjt_tiles[g][:, :, :], in_=adjt[:, 4 * g:4 * (g + 1), :]
            )
        # softmax-denominator ones column of the matmul rhs
        nc.vector.memset(rhs_sb[:, :, :, 64:65], 1.0)

        # ---- projection + head-0 attention, interleaved ----
        # The PE stream alternates projection blocks with head-0 attention
        # matmuls so neither the DVE nor the PE ever waits for the whole
        # other phase (in-order engine queues).
        psmain = ctx.enter_context(tc.tile_pool(name="psmain", bufs=1, space="PSUM"))
        pswide = tc.alloc_tile_pool(name="pswide", bufs=3, space="PSUM")

        def alloc_head_psums(h):
            # 4 row-blocks packed per PSUM bank; start=True clears the whole
            # bank's has_written bits, so only the bank's first matmul sets it
            # (writes to cleared regions overwrite, then accumulate)
            P0 = psmain.tile([128, 4, 65], f32, tag="P0", name=f"P0_{h}")
            P1 = psmain.tile([128, 4, 65], f32, tag="P1", name=f"P1_{h}")
            return P0, P1

        def emit_pair(h, jc0, P0, P1):
            # two j-chunks per mask op: TS scalars differ per chunk so the
            # fused mult+max stays per-chunk; the mask (elementwise min with
            # adj stored as {0, 60000}) runs once over [128, 2*RPC], halving
            # its fixed overhead.
            g, r = jc0 // 4, jc0 % 4
            t = ppool.tile([128, 2, RPC], f16, tag="t", name=f"t_{h}_{jc0}")
            for q in range(2):
                jc = jc0 + q
                nc.vector.tensor_scalar(
                    t[:, q, :], wbc_tiles[h][:, :],
                    A_sb[:, jc, h:h + 1], B_sb[:, jc, h:h + 1],
                    Alu.mult, Alu.max,
                )
            p = ppool.tile([128, 2, RPC], f16, tag="p", name=f"p_{h}_{jc0}")
            nc.vector.tensor_tensor(
                p[:, :, :], t[:, :, :], adjt_tiles[g][:, r:r + 2, :], Alu.min
            )
            for q in range(2):
                jc = jc0 + q
                for ib in range(IB):
                    P = P0 if ib < 4 else P1
                    nc.tensor.matmul(
                        P[:, ib % 4, :], p[:, q, ib * 128:(ib + 1) * 128],
                        rhs_sb[:, h, jc, :],
                        start=(jc == 0 and ib % 4 == 0),
                        stop=(jc == NB - 1 and ib % 4 == 3),
                        skip_group_check=True,
                    )

        def emit_epilogue(h, P0, P1):
            for ib in range(IB):
                P = (P0 if ib < 4 else P1)[:, ib % 4, :]
                rcp = epool.tile([128, 1], f32, tag="rcp", name=f"rcp{h}_{ib}")
                nc.vector.reciprocal(rcp, P[:, 64:65])
                rt = epool.tile([128, 64], f32, tag="rt", name=f"rt{h}_{ib}")
                nc.scalar.activation(rt, P[:, 0:64], Act.Relu, scale=rcp)
                qt = epool.tile([128, 64], f32, tag="qt", name=f"qt{h}_{ib}")
                nc.scalar.activation(qt, P[:, 0:64], Act.Exp, scale=rcp)
                # elu(v) = relu(v) - relu(1 - e^v)
                mt = epool.tile([128, 64], f32, tag="mt", name=f"mt{h}_{ib}")
                nc.scalar.activation(mt, qt, Act.Relu, scale=-1.0, bias=1.0)
                ot = opool.tile([128, 64], f32, tag="ot", name=f"ot{h}_{ib}")
                nc.vector.tensor_tensor(ot, rt, mt, Alu.subtract)
                nc.sync.dma_start(out=out_t[h, ib * 128:(ib + 1) * 128, :], in_=ot)

        def emit_wbc(h):
            # broadcast head h's w row to all 128 partitions via one-hot matmuls
            for b in range(IB):
                r = b * HPC + h
                bc_ps = pswide.tile([128, 128], f32, tag="bcps", bufs=2)
                nc.tensor.matmul(
                    bc_ps, onehot_sb[:, r, :], wct_sb[:, :], start=True, stop=True
                )
                nc.scalar.activation(
                    wbc_tiles[h][:, b * 128:(b + 1) * 128], bc_ps, Act.Copy
                )

        P0_0, P1_0 = alloc_head_psums(0)
        for nb in range(NB):
            pw = pswide.tile([128, 264], f32)
            for f in range(FB):
                nc.tensor.matmul(
                    pw, xhi_sb[:, f, nb * 128:(nb + 1) * 128], wext_sb[:, f, 0:264],
                    start=(f == 0), stop=False,
                )
            # all three double-fp16 sd terms accumulate into psum cols 256:264,
            # so sd needs no post-add at all
            for f in range(FB):
                nc.tensor.matmul(
                    pw[:, 256:264], xhi_sb[:, f, nb * 128:(nb + 1) * 128],
                    wext_sb[:, f, 264:272], start=False, stop=False,
                )
            for f in range(FB):
                nc.tensor.matmul(
                    pw[:, 256:264], xlo_sb[:, f, nb * 128:(nb + 1) * 128],
                    wext_sb[:, f, 256:264], start=False, stop=(f == FB - 1),
                )
            # sd first (the w/A/B chain is on the critical path), then rhs
            nc.scalar.activation(sd_sb[:, nb, :], pw[:, 256:264], Act.Copy)
            dcols = sd_sb[:, nb:nb + 1, 1:8:2]
            nc.scalar.activation(A_sb[:, nb:nb + 1, :], dcols, Act.Exp)
            nc.scalar.activation(
                B_sb[:, nb:nb + 1, :], dcols, Act.Exp,
                scale=ALPHA, bias=scbias[:, :],
            )
            nc.scalar.activation(
                rhs_sb[:, :, nb, 0:64],
                pw[:, 0:256].rearrange("p (h d) -> p h d", h=HPC),
                Act.Copy,
            )
            if nb == IB - 1:
                # own rows (blocks 0..7 thanks to the permutation): w factors,
                # transpose to one free-dim row — no DMA (a DMA here starves
                # behind the bulk input stream)
                nc.scalar.activation(
                    wcol_sb[:, :, :], sd_sb[:, 0:IB, 0:8:2],
                    Act.Exp, scale=1.0 - ALPHA, bias=scbias[:, :],
                )
                wct_ps = pswide.tile([32, 128], f16, tag="wctps", bufs=1)
                nc.tensor.transpose(
                    wct_ps, wcol_sb[:, :, :].rearrange("p a b -> p (a b)"),
                    ident_sb[:, :],
                )
                nc.scalar.activation(wct_sb[:, :], wct_ps[:, :], Act.Copy)
                emit_wbc(0)
            if nb >= IB - 1 and (nb - (IB - 1)) % 5 == 4:
                # stagger the other heads' broadcasts so they don't delay
                # the projection evacuations
                hh = (nb - (IB - 1)) // 5 + 1
                if hh < HPC:
                    emit_wbc(hh)
            if nb >= IB and nb % 2 == 1:
                # head-0 attention, IB chunks behind the projection: its
                # wbc/rhs/A/B producers must already be emitted (trace order
                # is program order)
                emit_pair(0, nb - IB - 1, P0_0, P1_0)
        pswide.release()
        for jc in range(NB - IB, NB, 2):
            emit_pair(0, jc, P0_0, P1_0)
        emit_epilogue(0, P0_0, P1_0)

        # ---- remaining heads ----
        for h in range(1, HPC):
            P0, P1 = alloc_head_psums(h)
            for jc in range(0, NB, 2):
                emit_pair(h, jc, P0, P1)
            emit_epilogue(h, P0, P1)
    nc.finalize()
    return nc


def _get_nc():
    if "nc" not in _CACHE:
        _CACHE["nc"] = _build()
    return _CACHE["nc"]


def _prepare_in_maps(x, adj, W, a):
    x = np.asarray(x, np.float32)
    adj = np.asarray(adj, np.float32)
    W = np.asarray(W, np.float32)
    a = np.asarray(a, np.float32)
    xT = np.ascontiguousarray(x.T)
    adjT = np.ascontiguousarray(adj.T)
    all_rows = np.arange(N)
    in_maps = []
    for c in range(NCORES):
        hg, rg = divmod(c, RG)
        own = np.arange(rg * RPC, (rg + 1) * RPC)
        perm = np.concatenate([own, np.delete(all_rows, own)])
        xt = xT[:, perm]
        xhi = xt.astype(np.float16)
        xlo = (xt - xhi.astype(np.float32)).astype(np.float16)
        heads = [hg * HPC + h for h in range(HPC)]
        wsd = np.stack(
            sum([[W[gh] @ a[gh, :HID], W[gh] @ a[gh, HID:]] for gh in heads], []),
            axis=1,
        ).astype(np.float32)  # [F, 8] cols (h0 s, h0 d, h1 s, ...)
        wsdh = wsd.astype(np.float16)
        wsdl = (wsd - wsdh.astype(np.float32)).astype(np.float16)
        wext = np.concatenate(
            [W[gh] for gh in heads] + [wsdh, wsdl], axis=1
        ).astype(np.float16)  # [F, 272]
        adjt_c = (adjT[perm][:, own] * 60000.0).astype(np.float16)

        def swz(m):
            # [K*128, M] -> [128, K, M]: partition-major so each SBUF
            # partition reads one contiguous DRAM run
            k = m.shape[0] // 128
            return np.ascontiguousarray(
                m.reshape(k, 128, m.shape[1]).transpose(1, 0, 2)
            )

        in_maps.append({
            "xhi": swz(xhi),
            "xlo": swz(xlo),
            "wext": swz(wext),
            "adjt": swz(adjt_c),
        })
    return in_maps


def _assemble(results):
    full = np.empty((N, H * HID), np.float32)
    for c in range(NCORES):
        hg, rg = divmod(c, RG)
        o = results[c]["out"]  # [HPC, RPC, HID]
        full[rg * RPC:(rg + 1) * RPC, hg * HPC * HID:(hg + 1) * HPC * HID] = (
            o.transpose(1, 0, 2).reshape(RPC, HPC * HID)
        )
    return full


def _run(in_maps, **kw):
    return run_bass_kernel_spmd(_get_nc(), in_maps, list(range(NCORES)), **kw)


def kernel(x, adj, W, a):
    in_maps = _prepare_in_maps(x, adj, W, a)
    res = _run(in_maps)
    return _assemble(res.results)
